# revision 16
# baseline (speedup 1.0000x reference)
"""Euclidean distance (cdist) kernel for Trainium2, 8 NeuronCores.

out[b, j] = || x[b, :] - weight[:, j] ||_2   for x [4096, 64], weight [64, 50000].

Sharding (per hint): K = 50000 split into 8 slabs of 6250, one per core
(tensor-parallel over prototypes); x replicated; no cross-core reduction.

Math: dist^2 = x2[b] + w2[j] - 2*x@w, fused into ONE fp16 matmul by
augmenting the contraction dim (D=64 of 128 partitions free):

  lhsT = [-2x^T; 1; 1]  [66, B]   rhs = [w; w2_hi; w2_lo]  [66, KS]
  PSUM = -2 x@w + w2              (fp32 accumulation)
  ScalarE: out = sqrt(PSUM + x2[b])  (x2 as exact per-partition bias).

fp16 operands carry 11 mantissa bits -- the same precision as the PE's
fp32r internal rounding (emulated max rel err 1.5e-4 vs the 2e-2 gate;
w2 is kept exact via a Dekker hi/lo pair riding two ones-rows) -- but
run the fast 1-cycle/col matmul path with hardware fast-weight-load,
and halve the input DMA bytes.

The kernel is HBM-store-bound: 102.4 MB of fp32 output per core vs
~1.4 MB of inputs. The store stream must never idle:
  - ALL stores go on one HWDGE ring (SP via nc.sync): a single
    sequential HBM write stream sustains ~360 GB/s, while two
    concurrent rings drop to ~317 GB/s (the SDMA engines pay a
    descriptor-refetch on every packet-granular ring switch);
  - outp bufs=4 lets the scalar engine run 3 tiles ahead of the store
    stream, so every store instruction is dispatched (and its
    descriptors queued on the ring) long before the previous transfer
    drains -- pre-queued instructions stream back-to-back with no
    completion-latency gap;
  - loads go on the SWDGE (gpsimd) ring, off the store path;
  - b-tile 0 stores per 512-col j-tile (first store dispatches right
    after the first matmul+sqrt), b-tiles 1-3 store per 2048-col chunk
    (keeps the ring fed while the compute pipeline fills and the PE
    warms up), b-tiles 4+ use one contiguous 3.2 MB store each.
Per core: 32 b-tiles of 128 rows; per b-tile 13 matmuls of <=512 cols
into 2048-col PSUM tiles.
"""

import numpy as np
from contextlib import ExitStack

import concourse.bass as bass
import concourse.bacc as bacc
import concourse.tile as tile
from concourse import mybir
from concourse.bass_utils import run_bass_kernel_spmd

B, D, K = 4096, 64, 50000
NCORES = 8
KS = K // NCORES  # 6250 columns per core
P = 128
JT = 512          # matmul free-dim tile (one PSUM bank of fp32)
DL = D + 2        # 66: contraction rows ([-2x; 1; 1] vs [w; w2_hi; w2_lo])

F32 = mybir.dt.float32
F16 = mybir.dt.float16


def build_nc(b=B, ks=KS):
    nbt = b // P
    nc = bacc.Bacc("TRN2", target_bir_lowering=False, debug=False)
    xst = nc.dram_tensor("xst", [DL, b], F16, kind="ExternalInput").ap()
    wst = nc.dram_tensor("wst", [DL, ks], F16, kind="ExternalInput").ap()
    x2 = nc.dram_tensor("x2", [P, nbt], F32, kind="ExternalInput").ap()
    out = nc.dram_tensor("out", [b, ks], F32, kind="ExternalOutput").ap()

    CHUNK = 4 * JT  # 2048: one 4-bank PSUM tile, one ACT instruction
    chunks = [(c0, min(CHUNK, ks - c0)) for c0 in range(0, ks, CHUNK)]

    with tile.TileContext(nc) as tc:
        with ExitStack() as ctx:
            singles = ctx.enter_context(tc.tile_pool(name="singles", bufs=1))
            outp = ctx.enter_context(tc.tile_pool(name="outp", bufs=4))
            psum = ctx.enter_context(tc.tile_pool(name="psum", bufs=2, space="PSUM"))

            wst_sb = singles.tile([DL, ks], F16)
            xst_sb = singles.tile([DL, b], F16)
            x2_sb = singles.tile([P, nbt], F32)

            # Dummy sqrt on a memset scratch: forces the scalar engine's
            # ACT_TABLE_LOAD (~1.3 us) to run during the load window
            # instead of on the critical path before the first real sqrt.
            warm = singles.tile([P, 8], F32)
            nc.vector.memset(warm[:, 0:4], 1.0)
            nc.scalar.activation(
                warm[:, 4:8], warm[:, 0:4],
                mybir.ActivationFunctionType.Sqrt, bias=0.0, scale=1.0,
            )

            # Scratch operand for the PE warm-up matmuls (see below).
            mmwarm = singles.tile([DL, JT], F16)
            nc.vector.memset(mmwarm, 1.0)
            NWARM = 10

            # Loads on the SWDGE (gpsimd) ring, off the store path and
            # good at packet-aggregating these small-per-partition
            # shapes (HWDGE sprays them into ~344 B packets). Order =
            # criticality: the first j-tile's weights + b-tile-0 x gate
            # the first matmul.
            nc.gpsimd.dma_start(out=wst_sb[:, 0:JT], in_=wst[:, 0:JT])
            nc.gpsimd.dma_start(out=xst_sb[:, 0:P], in_=xst[:, 0:P])
            nc.gpsimd.dma_start(out=x2_sb, in_=x2)
            nc.gpsimd.dma_start(out=wst_sb[:, JT:CHUNK], in_=wst[:, JT:CHUNK])
            nc.gpsimd.dma_start(out=xst_sb[:, P:b], in_=xst[:, P:b])
            nc.gpsimd.dma_start(out=wst_sb[:, CHUNK:ks], in_=wst[:, CHUNK:ks])

            for ib in range(nbt):
                # Store granularity: j-tile for b-tile 0, chunk for b-tiles
                # 1-7 (pipeline fill), whole 3.2 MB row after that.
                ot = outp.tile([P, ks], F32)
                row = out[ib * P:(ib + 1) * P, :]
                for ic, (c0, cn) in enumerate(chunks):
                    pt = psum.tile([P, CHUNK], F32)
                    if ib == 0 and ic == 0:
                        # PE warm-up: a dummy accumulation group into this
                        # tile's first bank during the load window keeps
                        # the PE active >4 us so the HAM throttle ramps it
                        # to full clock before the real matmuls (the
                        # pipeline-fill phase is compute-marginal on a
                        # cold PE). The in-order PE retires the group
                        # before chunk 0's start=True matmul overwrites
                        # the region, so it needs no space or semaphores.
                        for i in range(NWARM):
                            nc.tensor.matmul(
                                pt[:, 0:JT], mmwarm[:, 0:P], mmwarm[:, 0:JT],
                                start=(i == 0), stop=(i == NWARM - 1),
                            )
                    for jj in range(0, cn, JT):
                        jn = min(JT, cn - jj)
                        nc.tensor.matmul(
                            pt[:, jj:jj + jn],
                            xst_sb[:, ib * P:(ib + 1) * P],
                            wst_sb[:, c0 + jj:c0 + jj + jn],
                            start=True,
                            stop=True,
                        )
                        if ib == 0:
                            nc.scalar.activation(
                                ot[:, c0 + jj:c0 + jj + jn],
                                pt[:, jj:jj + jn],
                                mybir.ActivationFunctionType.Sqrt,
                                bias=x2_sb[:, ib:ib + 1],
                                scale=1.0,
                            )
                            nc.sync.dma_start(
                                out=row[:, c0 + jj:c0 + jj + jn],
                                in_=ot[:, c0 + jj:c0 + jj + jn],
                            )
                    if ib > 0:
                        nc.scalar.activation(
                            ot[:, c0:c0 + cn],
                            pt[:, :cn],
                            mybir.ActivationFunctionType.Sqrt,
                            bias=x2_sb[:, ib:ib + 1],
                            scale=1.0,
                        )
                        if ib <= 7:
                            nc.sync.dma_start(
                                out=row[:, c0:c0 + cn], in_=ot[:, c0:c0 + cn]
                            )
                if ib > 7:
                    nc.sync.dma_start(out=row, in_=ot)
    nc.compile()
    return nc


def prep_inputs(x, weight):
    """Host-side prep: augmented-contraction fp16 operand matrices."""
    x = np.ascontiguousarray(x, dtype=np.float32)
    weight = np.ascontiguousarray(weight, dtype=np.float32)
    b, d = x.shape
    k = weight.shape[1]
    x2 = (x.astype(np.float64) ** 2).sum(axis=1).astype(np.float32)
    w2 = (weight.astype(np.float64) ** 2).sum(axis=0).astype(np.float32)
    w2_hi = w2.astype(np.float16)
    w2_lo = (w2 - w2_hi.astype(np.float32)).astype(np.float16)

    xst = np.empty((DL, b), dtype=np.float16)
    xst[:d] = (-2.0 * x).T.astype(np.float16)
    xst[d] = 1.0
    xst[d + 1] = 1.0
    wst = np.empty((DL, k), dtype=np.float16)
    wst[:d] = weight.astype(np.float16)
    wst[d] = w2_hi
    wst[d + 1] = w2_lo
    x2t = np.ascontiguousarray(x2.reshape(b // P, P).T)  # [P, NBT]
    return xst, wst, x2t


_nc_cache = {}


def _get_nc():
    if "nc" not in _nc_cache:
        _nc_cache["nc"] = build_nc()
    return _nc_cache["nc"]


def make_in_maps(x, weight, ks=KS):
    xst, wst, x2t = prep_inputs(x, weight)
    return [
        {"xst": xst,
         "wst": np.ascontiguousarray(wst[:, i * ks:(i + 1) * ks]),
         "x2": x2t}
        for i in range(NCORES)
    ]


def kernel(x, weight):
    nc = _get_nc()
    in_maps = make_in_maps(x, weight)
    res = run_bass_kernel_spmd(nc, in_maps, core_ids=list(range(NCORES)))
    return np.concatenate([res.results[i]["out"] for i in range(NCORES)], axis=1)
